# revision 14
# baseline (speedup 1.0000x reference)
"""Trainium2 Bass kernel for causal self-attention with RoPE.

Problem: B=2, S=2048, H=1024, NH=16 heads (HD=64), fp32, causal mask,
rotary embeddings, out = softmax(rope(XWq)(rope(XWk))^T/8 + mask) (XWv) Wo^T.

Sharding (8 cores): tensor-parallel over heads x data-parallel over batch.
core i -> (batch b = i//4, head-group g = i%4 of 4 heads = 256 channels).
Each core computes its group's Q/K/V projections, RoPE, causal attention and
a partial output projection (attnout_g @ Wo_g^T); the host sums the 4 group
partials per batch.

Device-side layout strategy (all matmuls fp32r = full PE rate, fp32 storage):
 - hidden states are fed TRANSPOSED (XT [H, S]) so every matmul contracts
   over the partition dim naturally.
 - Q^T/K^T [c, s] come straight out of the projection matmuls.
 - rotate_half is a constant signed-permutation matmul (R2 block-diag for 2
   heads per 128-partition tile); RoPE is then purely elementwise in [c, s].
 - scores are computed TRANSPOSED (P^T [ks, qs]); softmax uses exp without
   max-subtraction (scores are bounded ~ +-4 for this problem's scale-0.02
   weights) and a row-of-ones column appended to V gives the softmax
   denominator for free in the attn@V matmul.
 - attention output comes out transposed [hd, qs], which is exactly the lhsT
   the output projection wants.
 - walrus requires every fp32r-matmul input buffer to be *produced* as
   fp32r, so all matmul-feeding tiles are declared float32r and DMAs use
   bitcast APs (same bits; fp32r is stored as 4-byte fp32).
"""
import sys
import numpy as np

sys.path.insert(0, '/opt/trn_rl_repo')

B, S, H, NH, HD = 2, 2048, 1024, 16, 64
GROUPS = 4            # head-groups (cores per batch)
HEADS_PER_CORE = 4
C = HEADS_PER_CORE * HD   # 256 channels per core
P = 128               # partitions
SC = 512              # s-chunk (matmul free dim)
N_SCHUNK = S // SC    # 4
N_HT = H // P         # 8 h-tiles
N_ST = S // P         # 16 s-tiles
ROPE_THETA = 10000.0

_PROGRAM_CACHE = {}
TRACE = False          # set True (e.g. from test.py) to profile and fill LAST_EXEC_NS
LAST_EXEC_NS = None


def _build_program():
    import concourse.bass as bass
    import concourse.mybir as mybir
    import concourse.tile as tile
    from concourse import bacc

    f32 = mybir.dt.float32
    f32r = mybir.dt.float32r
    Exp = mybir.ActivationFunctionType.Exp
    Copy = mybir.ActivationFunctionType.Copy
    mult = mybir.AluOpType.mult
    add = mybir.AluOpType.add

    nc = bacc.Bacc("TRN2", target_bir_lowering=False, debug=False, num_devices=8)

    # ---- DRAM parameters (per-core inputs) ----
    hsT = nc.declare_dram_parameter("hsT", [H, S], f32, isOutput=False)
    wqT = nc.declare_dram_parameter("wqT", [H, C], f32, isOutput=False)
    wkT = nc.declare_dram_parameter("wkT", [H, C], f32, isOutput=False)
    wvT = nc.declare_dram_parameter("wvT", [H, C], f32, isOutput=False)
    woT = nc.declare_dram_parameter("woT", [C, H], f32, isOutput=False)
    cosT = nc.declare_dram_parameter("cosT", [P, S], f32, isOutput=False)
    sinT = nc.declare_dram_parameter("sinT", [P, S], f32, isOutput=False)
    masks = nc.declare_dram_parameter("masks", [P, 4 * SC], f32, isOutput=False)
    r2t = nc.declare_dram_parameter("r2t", [P, P], f32, isOutput=False)
    onesv = nc.declare_dram_parameter("onesv", [1, P], f32, isOutput=False)
    out_d = nc.declare_dram_parameter("out", [S, H], f32, isOutput=True)

    mm = nc.tensor.matmul

    with tile.TileContext(nc) as tc, \
         nc.allow_low_precision(reason="fp32r matmuls; accumulation stays fp32 in PSUM"):
        # ---------------- persistent tiles ----------------
        import contextlib
        stack = contextlib.ExitStack()
        persist = stack.enter_context(tc.tile_pool(name="persist", bufs=1))

        cos_sb = persist.tile([P, S], f32, tag="cos")
        sin_sb = persist.tile([P, S], f32, tag="sin")
        masks_sb = persist.tile([P, 4 * SC], f32r, tag="masks")
        r2t_sb = persist.tile([P, P], f32r, tag="r2t")
        ones_sb = persist.tile([1, P], f32r, tag="ones")
        wo_sb = [persist.tile([P, H], f32r, tag=f"wo{i}", name=f"wo{i}") for i in range(2)]
        qr_sb = [persist.tile([P, S], f32r, tag=f"qr{i}", name=f"qr{i}") for i in range(2)]
        kr_sb = [persist.tile([P, S], f32r, tag=f"kr{i}", name=f"kr{i}") for i in range(2)]
        vx_sb = [persist.tile([P, HEADS_PER_CORE, HD + 1], f32r, tag=f"vx{i}", name=f"vx{i}")
                 for i in range(N_ST)]
        at_sb = [persist.tile([P, S], f32r, tag=f"at{i}", name=f"at{i}") for i in range(2)]

        nc.sync.dma_start(out=cos_sb[:], in_=cosT[:])
        nc.sync.dma_start(out=sin_sb[:], in_=sinT[:])
        nc.sync.dma_start(out=masks_sb[:], in_=masks.ap().bitcast(f32r))
        nc.sync.dma_start(out=r2t_sb[:], in_=r2t.ap().bitcast(f32r))
        nc.sync.dma_start(out=ones_sb[:], in_=onesv.ap().bitcast(f32r))
        for i in range(2):
            nc.sync.dma_start(out=wo_sb[i][:],
                              in_=woT[i * P:(i + 1) * P, :].bitcast(f32r))

        # ---------------- phase 1: projections + rope ----------------
        with tc.tile_pool(name="ph1", bufs=1) as ph1, \
             tc.tile_pool(name="qtmp", bufs=3) as qtmp_pool, \
             tc.tile_pool(name="rtmp", bufs=3) as rtmp_pool, \
             tc.tile_pool(name="ps1", bufs=1, space="PSUM") as ps1:

            xt = [ph1.tile([P, S], f32r, tag=f"xt{t}", name=f"xt{t}") for t in range(N_HT)]
            w_sb = {}
            for name, dram in (("q", wqT), ("k", wkT), ("v", wvT)):
                w_sb[name] = ph1.tile([P, N_HT, C], f32r, tag=f"w{name}", name=f"w{name}")
                nc.sync.dma_start(
                    out=w_sb[name][:],
                    in_=dram.rearrange("(t p) c -> p t c", p=P).bitcast(f32r))
            for j in range(N_SCHUNK):
                for t in range(N_HT):
                    nc.sync.dma_start(
                        out=xt[t][:, j * SC:(j + 1) * SC],
                        in_=hsT[t * P:(t + 1) * P, j * SC:(j + 1) * SC].bitcast(f32r))

            # Q^T and K^T -> rope -> qr_sb / kr_sb
            for j in range(N_SCHUNK):
                ssl = slice(j * SC, (j + 1) * SC)
                for (wname, dest) in (("q", qr_sb), ("k", kr_sb)):
                    for ct in range(2):
                        csl = slice(ct * P, (ct + 1) * P)
                        ps_q = ps1.tile([P, SC], f32, tag="psq", bufs=3)
                        for t in range(N_HT):
                            mm(ps_q[:], w_sb[wname][:, t, csl], xt[t][:, ssl],
                               start=(t == 0), stop=(t == N_HT - 1))
                        # raw projection to SBUF (needed as rot-matmul input)
                        qt = qtmp_pool.tile([P, SC], f32r, tag="qt")
                        nc.scalar.copy(qt[:], ps_q[:])
                        ps_rot = ps1.tile([P, SC], f32, tag="psrot", bufs=2)
                        mm(ps_rot[:], r2t_sb[:], qt[:], start=True, stop=True)
                        # rope combine: dest = qt*cos + rot*sin
                        dsl = dest[ct][:, ssl]
                        rt = rtmp_pool.tile([P, SC], f32, tag="rt")
                        nc.vector.tensor_tensor(rt[:], ps_rot[:], sin_sb[:, ssl], mult)
                        nc.vector.tensor_tensor(dsl, qt[:], cos_sb[:, ssl], mult)
                        nc.vector.tensor_tensor(dsl, dsl, rt[:], add)

            # V (natural layout, with ones column) -> vx_sb
            for st in range(N_ST):
                ps_v = ps1.tile([P, C], f32, tag="psv", bufs=3)
                ssl = slice(st * P, (st + 1) * P)
                for t in range(N_HT):
                    mm(ps_v[:], xt[t][:, ssl], w_sb["v"][:, t, :],
                       start=(t == 0), stop=(t == N_HT - 1))
                nc.scalar.copy(
                    vx_sb[st][:, :, 0:HD],
                    ps_v.rearrange("p (h d) -> p h d", d=HD))
                # ones column: Copy(in*0 + 1) from any live fp32 source
                nc.scalar.activation(
                    vx_sb[st][:, :, HD], ps_v[:, 0:HEADS_PER_CORE],
                    Copy, bias=1.0, scale=0.0)

        # ---------------- phase 2: attention + output projection ----------------
        with tc.tile_pool(name="pexp", bufs=4) as pexp_pool, \
             tc.tile_pool(name="small", bufs=4) as small_pool, \
             tc.tile_pool(name="outsb", bufs=2) as out_pool, \
             tc.tile_pool(name="ps2", bufs=1, space="PSUM") as ps2:

            for j in range(N_SCHUNK):
                q0 = j * SC
                qsl = slice(q0, q0 + SC)
                n_kt = (q0 + SC) // P  # ks-tiles 0..n_kt-1
                for h in range(HEADS_PER_CORE):
                    ct, hh = divmod(h, 2)
                    hsl = slice(hh * HD, (hh + 1) * HD)  # within the ct tile
                    ps_av = ps2.tile([HD + 1, SC], f32, tag="av", bufs=2)
                    for kt in range(n_kt):
                        k0 = kt * P
                        ps_s = ps2.tile([P, SC], f32, tag="pss", bufs=3)
                        mm(ps_s[:], kr_sb[ct][hsl, k0:k0 + P],
                           qr_sb[ct][hsl, qsl], start=True, stop=True)
                        pe = pexp_pool.tile([P, SC], f32r, tag="pe")
                        nc.scalar.activation(pe[:], ps_s[:], Exp, scale=0.125)
                        d = (k0 - q0) // P
                        if d >= 0:  # diagonal tile: apply causal 0/1 mask
                            nc.vector.tensor_tensor(
                                pe[:], pe[:], masks_sb[:, d * SC:(d + 1) * SC], mult)
                        mm(ps_av[:], vx_sb[kt][:, h, :], pe[:],
                           start=(kt == 0), stop=(kt == n_kt - 1))
                    # normalize by the ones-column sum and store to attnT
                    rec = small_pool.tile([1, SC], f32r, tag="rec")
                    nc.vector.reciprocal(rec[:], ps_av[HD:HD + 1, :])
                    ps_bc = ps2.tile([HD, SC], f32, tag="bc", bufs=1)
                    mm(ps_bc[:], ones_sb[:, 0:HD], rec[:], start=True, stop=True)
                    rb = small_pool.tile([HD, SC], f32, tag="rb")
                    nc.vector.tensor_copy(rb[:], ps_bc[:])
                    nc.vector.tensor_tensor(
                        at_sb[ct][hsl, qsl], ps_av[0:HD, :], rb[:], mult)

                # output projection for this chunk's 4 s-tiles
                for st in range(4 * j, 4 * j + 4):
                    osb = out_pool.tile([P, H], f32, tag="osb")
                    ssl = slice(st * P, (st + 1) * P)
                    for oc in range(2):
                        ps_o = ps2.tile([P, SC], f32, tag="pso", bufs=2)
                        for ct in range(2):
                            mm(ps_o[:], at_sb[ct][:, ssl],
                               wo_sb[ct][:, oc * SC:(oc + 1) * SC],
                               start=(ct == 0), stop=(ct == 1))
                        nc.scalar.copy(osb[:, oc * SC:(oc + 1) * SC], ps_o[:])
                    nc.sync.dma_start(out=out_d[ssl, :], in_=osb[:])

        stack.close()

    nc.compile()
    return nc


def _get_program():
    if "nc" not in _PROGRAM_CACHE:
        _PROGRAM_CACHE["nc"] = _build_program()
    return _PROGRAM_CACHE["nc"]


def _host_consts(position_ids_row):
    inv_freq = 1.0 / (ROPE_THETA ** (np.arange(0, HD, 2, dtype=np.float32) / HD))
    t = position_ids_row.astype(np.float32)
    freqs = t[None, :] * inv_freq[(np.arange(P) % (HD // 2))][:, None]  # [128, S]
    cosT = np.cos(freqs).astype(np.float32)
    sinT = np.sin(freqs).astype(np.float32)
    return np.ascontiguousarray(cosT), np.ascontiguousarray(sinT)


def _make_r2t():
    R = np.zeros((HD, HD), dtype=np.float32)
    for j in range(HD // 2):
        R[j, j + HD // 2] = -1.0
        R[j + HD // 2, j] = 1.0
    R2 = np.zeros((P, P), dtype=np.float32)
    R2[:HD, :HD] = R
    R2[HD:, HD:] = R
    return np.ascontiguousarray(R2.T)


def _make_masks():
    # masks[p, d*SC + jj] = 1 if jj >= p + d*128 (causal, qs >= ks)
    m = np.zeros((P, 4, SC), dtype=np.float32)
    jj = np.arange(SC)[None, None, :]
    p = np.arange(P)[:, None, None]
    d = np.arange(4)[None, :, None]
    m[:] = (jj >= p + d * P).astype(np.float32)
    return np.ascontiguousarray(m.reshape(P, 4 * SC))


def kernel(**inputs):
    from concourse.bass_utils import run_bass_kernel_spmd

    hs = np.asarray(inputs["hidden_states"], dtype=np.float32)
    wq = np.asarray(inputs["wq"], dtype=np.float32)
    wk = np.asarray(inputs["wk"], dtype=np.float32)
    wv = np.asarray(inputs["wv"], dtype=np.float32)
    wo = np.asarray(inputs["wo"], dtype=np.float32)
    pos = np.asarray(inputs["position_ids"])

    nc = _get_program()

    wqT = np.ascontiguousarray(wq.T)
    wkT = np.ascontiguousarray(wk.T)
    wvT = np.ascontiguousarray(wv.T)
    woT = np.ascontiguousarray(wo.T)
    r2t = _make_r2t()
    masks = _make_masks()
    onesv = np.ones((1, P), dtype=np.float32)

    hsT = [np.ascontiguousarray(hs[b].T) for b in range(B)]
    tables = [_host_consts(pos[b]) for b in range(B)]

    in_maps = []
    for core in range(8):
        b, g = divmod(core, GROUPS)
        csl = slice(g * C, (g + 1) * C)
        cosT, sinT = tables[b]
        in_maps.append(dict(
            hsT=hsT[b],
            wqT=np.ascontiguousarray(wqT[:, csl]),
            wkT=np.ascontiguousarray(wkT[:, csl]),
            wvT=np.ascontiguousarray(wvT[:, csl]),
            woT=np.ascontiguousarray(woT[csl, :]),
            cosT=cosT, sinT=sinT, masks=masks, r2t=r2t, onesv=onesv,
        ))

    res = run_bass_kernel_spmd(nc, in_maps, core_ids=list(range(8)), trace=TRACE)
    global LAST_EXEC_NS
    LAST_EXEC_NS = res.exec_time_ns
    out = np.zeros((B, S, H), dtype=np.float32)
    for core in range(8):
        b = core // GROUPS
        out[b] += res.results[core]["out"]
    return out


# revision 21
# speedup vs baseline: 6524.6066x; 6524.6066x over previous
"""Trainium2 Bass kernel for causal self-attention with RoPE.

Problem: B=2, S=2048, H=1024, NH=16 heads (HD=64), fp32, causal mask,
rotary embeddings, out = softmax(rope(XWq)(rope(XWk))^T/8 + mask) (XWv) Wo^T.

Sharding (8 cores): tensor-parallel over heads x data-parallel over batch.
core i -> (batch b = i//4, head-group g = i%4 of 4 heads = 256 channels).
Each core computes its group's Q/K/V projections, RoPE, causal attention and
a partial output projection (attnout_g @ Wo_g^T); the host sums the 4 group
partials per batch.

Device-side layout strategy (all matmuls fp32r = full PE rate, fp32 storage):
 - hidden states are fed TRANSPOSED (XT [H, S]) so every matmul contracts
   over the partition dim naturally.
 - Q^T/K^T [c, s] come straight out of the projection matmuls.
 - rotate_half is a constant signed-permutation matmul (R2 block-diag for 2
   heads per 128-partition tile); RoPE is then purely elementwise in [c, s].
 - scores are computed TRANSPOSED (P^T [ks, qs]); softmax uses exp without
   max-subtraction (scores are bounded ~ +-4 for this problem's scale-0.02
   weights) and a row-of-ones column appended to V gives the softmax
   denominator for free in the attn@V matmul.
 - attention output comes out transposed [hd, qs], which is exactly the lhsT
   the output projection wants.
 - walrus requires every fp32r-matmul input buffer to be *produced* as
   fp32r, so all matmul-feeding tiles are declared float32r and DMAs use
   bitcast APs (same bits; fp32r is stored as 4-byte fp32).
"""
import sys
import numpy as np

sys.path.insert(0, '/opt/trn_rl_repo')

B, S, H, NH, HD = 2, 2048, 1024, 16, 64
GROUPS = 4            # head-groups (cores per batch)
HEADS_PER_CORE = 4
C = HEADS_PER_CORE * HD   # 256 channels per core
P = 128               # partitions
SC = 512              # s-chunk (matmul free dim)
N_SCHUNK = S // SC    # 4
N_HT = H // P         # 8 h-tiles
N_ST = S // P         # 16 s-tiles
ROPE_THETA = 10000.0

_PROGRAM_CACHE = {}
TRACE = False          # set True (e.g. from test.py) to profile and fill LAST_EXEC_NS
LAST_EXEC_NS = None


def _build_program(loop_n=None):
    import concourse.bass as bass
    import concourse.mybir as mybir
    import concourse.tile as tile
    from concourse import bacc

    f32 = mybir.dt.float32
    f32r = mybir.dt.float32r
    Exp = mybir.ActivationFunctionType.Exp
    mult = mybir.AluOpType.mult
    add = mybir.AluOpType.add

    nc = bacc.Bacc("TRN2", target_bir_lowering=False, debug=False, num_devices=8)

    # ---- DRAM parameters (per-core inputs) ----
    hsT = nc.declare_dram_parameter("hsT", [H, S], f32, isOutput=False)
    wqT = nc.declare_dram_parameter("wqT", [H, C], f32, isOutput=False)
    wkT = nc.declare_dram_parameter("wkT", [H, C], f32, isOutput=False)
    wvT = nc.declare_dram_parameter("wvT", [H, C], f32, isOutput=False)
    woT = nc.declare_dram_parameter("woT", [C, H], f32, isOutput=False)
    cosT = nc.declare_dram_parameter("cosT", [P, S], f32, isOutput=False)
    sinT = nc.declare_dram_parameter("sinT", [P, S], f32, isOutput=False)
    masks = nc.declare_dram_parameter("masks", [P, 4 * SC], f32, isOutput=False)
    r2t = nc.declare_dram_parameter("r2t", [P, P], f32, isOutput=False)
    onesv = nc.declare_dram_parameter("onesv", [1, P], f32, isOutput=False)
    ones2 = nc.declare_dram_parameter("ones2", [2, P], f32, isOutput=False)
    vones = nc.declare_dram_parameter("vones", [P, HEADS_PER_CORE], f32, isOutput=False)
    out_d = nc.declare_dram_parameter("out", [S, H], f32, isOutput=True)

    mm = nc.tensor.matmul

    with tile.TileContext(nc) as tc, \
         nc.allow_low_precision(reason="fp32r matmuls; accumulation stays fp32 in PSUM"):
        import contextlib
        stack = contextlib.ExitStack()
        persist = stack.enter_context(tc.tile_pool(name="persist", bufs=1))
        work = stack.enter_context(tc.tile_pool(name="work", bufs=1))
        psp = stack.enter_context(tc.tile_pool(name="psp", bufs=1, space="PSUM"))

        # ---------------- persistent tiles ----------------
        cos_sb = persist.tile([P, S], f32, tag="cos")
        sin_sb = persist.tile([P, S], f32, tag="sin")
        masks_sb = persist.tile([P, 4 * SC], f32r, tag="masks")
        r2t_sb = persist.tile([P, P], f32r, tag="r2t")
        ones_sb = persist.tile([1, P], f32r, tag="ones")
        wo_sb = [persist.tile([P, H], f32r, tag=f"wo{i}", name=f"wo{i}") for i in range(2)]
        qr_sb = [persist.tile([P, S], f32r, tag=f"qr{i}", name=f"qr{i}") for i in range(2)]
        kr_sb = [persist.tile([P, S], f32r, tag=f"kr{i}", name=f"kr{i}") for i in range(2)]
        vx_sb = [persist.tile([P, HEADS_PER_CORE, HD + 1], f32r, tag=f"vx{i}", name=f"vx{i}")
                 for i in range(N_ST)]
        at_sb = [persist.tile([P, S], f32r, tag=f"at{i}", name=f"at{i}") for i in range(2)]
        w_sb = {n: work.tile([P, N_HT, C], f32r, tag=f"w{n}", name=f"w{n}")
                for n in ("q", "k", "v")}

        xt_tiles = {}

        def load_xt(j):
            # rotating per-h-tile chunk tiles (bufs=2)
            xt = []
            for t in range(N_HT):
                x = work.tile([P, SC], f32r, tag=f"xt{t}", name=f"xt{t}_{j}", bufs=2)
                nc.sync.dma_start(
                    out=x[:],
                    in_=hsT[t * P:(t + 1) * P, j * SC:(j + 1) * SC].bitcast(f32r))
                xt.append(x)
            xt_tiles[j] = xt

        # ---- startup DMAs: weights + first chunk interleaved, then consts ----
        wdram = {"q": wqT, "k": wkT, "v": wvT}
        def emit_startup_dmas():
            for t in range(N_HT):
                for n in ("q", "k", "v"):
                    nc.sync.dma_start(
                        out=w_sb[n][:, t, :],
                        in_=wdram[n][t * P:(t + 1) * P, :].bitcast(f32r))
                x = work.tile([P, SC], f32r, tag=f"xt{t}", name=f"xt{t}_0", bufs=2)
                nc.sync.dma_start(out=x[:], in_=hsT[t * P:(t + 1) * P, 0:SC].bitcast(f32r))
                xt_tiles.setdefault(0, []).append(x)
            nc.sync.dma_start(out=r2t_sb[:], in_=r2t.ap().bitcast(f32r))
            nc.sync.dma_start(out=cos_sb[:], in_=cosT[:])
            nc.sync.dma_start(out=sin_sb[:], in_=sinT[:])
            load_xt(1)
            nc.sync.dma_start(out=masks_sb[:], in_=masks.ap().bitcast(f32r))
            nc.sync.dma_start(out=ones_sb[:], in_=onesv.ap().bitcast(f32r))
            for st in range(N_ST):
                nc.sync.dma_start(out=vx_sb[st][:, :, HD],
                                  in_=vones.ap().bitcast(f32r))
            for i in range(2):
                nc.sync.dma_start(out=wo_sb[i][:],
                                  in_=woT[i * P:(i + 1) * P, :].bitcast(f32r))

        # ---------------- task generators ----------------
        def prep_tasks(j):
            """QK projections + rope + V projection for chunk j, as small tasks."""
            if j >= N_SCHUNK:
                return []
            ssl = slice(j * SC, (j + 1) * SC)
            tasks = []
            state = {}

            def mk_proj(wname, dest, ct):
                def t_first():
                    xt = xt_tiles[j]
                    csl = slice(ct * P, (ct + 1) * P)
                    ps_q = psp.tile([P, SC], f32, tag="ps512", name="psq", bufs=3)
                    state[(wname, ct)] = ps_q
                    for t in range(4):
                        mm(ps_q[:], w_sb[wname][:, t, csl], xt[t][:],
                           start=(t == 0), stop=False)

                def t_second():
                    xt = xt_tiles[j]
                    csl = slice(ct * P, (ct + 1) * P)
                    ps_q = state[(wname, ct)]
                    for t in range(4, N_HT):
                        mm(ps_q[:], w_sb[wname][:, t, csl], xt[t][:],
                           start=False, stop=(t == N_HT - 1))

                def t_rope():
                    ps_q = state[(wname, ct)]
                    qt = work.tile([P, SC], f32r, tag="qt", name="qt", bufs=2)
                    nc.scalar.copy(qt[:], ps_q[:])
                    ps_rot = psp.tile([P, SC], f32, tag="ps512", name="psrot", bufs=3)
                    mm(ps_rot[:], r2t_sb[:], qt[:], start=True, stop=True)
                    dsl = dest[ct][:, ssl]
                    rt = work.tile([P, SC], f32, tag="rt", name="rt", bufs=2)
                    nc.vector.tensor_tensor(rt[:], ps_rot[:], sin_sb[:, ssl], mult)
                    nc.gpsimd.tensor_tensor(dsl, qt[:], cos_sb[:, ssl], mult)
                    nc.gpsimd.tensor_tensor(dsl, dsl, rt[:], add)

                return [t_first, t_second, t_rope]

            for (wname, dest) in (("q", qr_sb), ("k", kr_sb)):
                for ct in range(2):
                    tasks.extend(mk_proj(wname, dest, ct))

            def mk_v(st):
                def t_vfirst():
                    xt = xt_tiles[j]
                    ps_v = psp.tile([P, C], f32, tag="ps512", name="psv", bufs=3)
                    state[("v", st)] = ps_v
                    lsl = slice((st - 4 * j) * P, (st - 4 * j) * P + P)
                    for t in range(4):
                        mm(ps_v[:], xt[t][:, lsl], w_sb["v"][:, t, :],
                           start=(t == 0), stop=False)

                def t_vsecond():
                    xt = xt_tiles[j]
                    ps_v = state[("v", st)]
                    lsl = slice((st - 4 * j) * P, (st - 4 * j) * P + P)
                    for t in range(4, N_HT):
                        mm(ps_v[:], xt[t][:, lsl], w_sb["v"][:, t, :],
                           start=False, stop=(t == N_HT - 1))
                    nc.vector.tensor_copy(
                        vx_sb[st][:, :, 0:HD],
                        ps_v.rearrange("p (h d) -> p h d", d=HD))

                return [t_vfirst, t_vsecond]

            for st in range(4 * j, 4 * j + 4):
                tasks.extend(mk_v(st))
            return tasks

        def attn_tasks(j):
            """Attention kt-group + normalization tasks for chunk j."""
            ssl = slice(j * SC, (j + 1) * SC)
            q0 = j * SC
            n_kt = (q0 + SC) // P
            tasks = []

            for ct in range(2):
                state = {}

                def mk_alloc(ct=ct, state=state):
                    def t_alloc():
                        state["av"] = [
                            psp.tile([HD + 1, SC], f32, tag="av",
                                     name=f"av{hh}", bufs=2)
                            for hh in range(2)]
                    return t_alloc

                def mk_kt(kt, ct=ct, state=state):
                    def t_kt():
                        k0 = kt * P
                        d = (k0 - q0) // P
                        c0 = max(d, 0) * P
                        pes = []
                        for hh in range(2):
                            hsl = slice(hh * HD, (hh + 1) * HD)
                            ps_s = psp.tile([P, SC], f32, tag="pss",
                                            name="pss", bufs=3)
                            mm(ps_s[:, c0:], kr_sb[ct][hsl, k0:k0 + P],
                               qr_sb[ct][hsl, q0 + c0:q0 + SC],
                               start=True, stop=True)
                            pe = work.tile([P, SC], f32r, tag="pe", name="pe",
                                           bufs=4)
                            nc.scalar.activation(pe[:, c0:], ps_s[:, c0:], Exp,
                                                 scale=0.125)
                            if d >= 0:
                                nc.gpsimd.tensor_tensor(
                                    pe[:, c0:], pe[:, c0:],
                                    masks_sb[:, d * SC + c0:(d + 1) * SC], mult)
                            pes.append(pe)
                        for hh in range(2):
                            h = 2 * ct + hh
                            mm(state["av"][hh][:, c0:], vx_sb[kt][:, h, :],
                               pes[hh][:, c0:],
                               start=(kt == 0), stop=(kt == n_kt - 1))
                    return t_kt

                def mk_norm(hh, ct=ct, state=state):
                    def t_norm():
                        hsl = slice(hh * HD, (hh + 1) * HD)
                        ps_av = state["av"][hh]
                        rec = work.tile([1, SC], f32r, tag="rec", name="rec",
                                        bufs=2)
                        nc.vector.reciprocal(rec[:], ps_av[HD:HD + 1, :])
                        ps_bc = psp.tile([HD, SC], f32, tag="ps512",
                                         name="psbc", bufs=3)
                        mm(ps_bc[:], ones_sb[:, 0:HD], rec[:],
                           start=True, stop=True)
                        rb = work.tile([HD, SC], f32, tag="rb", name="rb",
                                       bufs=2)
                        nc.vector.tensor_copy(rb[:], ps_bc[:])
                        nc.vector.tensor_tensor(
                            at_sb[ct][hsl, ssl], ps_av[0:HD, :], rb[:], mult)
                    return t_norm

                grp = [mk_alloc()]
                for kt in range(n_kt):
                    grp.append(mk_kt(kt))
                grp.append(mk_norm(0))
                grp.append(mk_norm(1))
                tasks.extend(grp)
            return tasks

        def out_tasks(j):
            tasks = []

            def mk_out(st):
                def t_out():
                    osb = work.tile([P, H], f32, tag="osb", name="osb", bufs=2)
                    osl = slice(st * P, (st + 1) * P)
                    for oc in range(2):
                        ps_o = psp.tile([P, SC], f32, tag="ps512", name="pso",
                                        bufs=3)
                        for ct in range(2):
                            mm(ps_o[:], at_sb[ct][:, osl],
                               wo_sb[ct][:, oc * SC:(oc + 1) * SC],
                               start=(ct == 0), stop=(ct == 1))
                        nc.vector.tensor_copy(osb[:, oc * SC:(oc + 1) * SC],
                                              ps_o[:])
                    nc.sync.dma_start(out=out_d[osl, :], in_=osb[:])
                return t_out

            for st in range(4 * j, 4 * j + 4):
                tasks.append(mk_out(st))
            return tasks

        def weave(stallers, fillers):
            """Emit stallers in order, interleaving fillers evenly between them."""
            nf, ns = len(fillers), len(stallers)
            fi = 0
            for si, t in enumerate(stallers):
                t()
                while fi < nf and (fi + 1) / nf <= (si + 1) / ns:
                    fillers[fi]()
                    fi += 1
            while fi < nf:
                fillers[fi]()
                fi += 1

        def body():
            # chunk 0 projections (nothing to overlap with yet)
            for t in prep_tasks(0):
                t()
            for j in range(N_SCHUNK):
                fillers = []
                if j + 1 < N_SCHUNK:
                    if j + 2 < N_SCHUNK:
                        fillers.append(lambda jj=j + 2: load_xt(jj))
                    fillers.extend(prep_tasks(j + 1))
                weave(attn_tasks(j) + out_tasks(j), fillers)

        if loop_n is None:
            emit_startup_dmas()
            body()
        else:
            with tc.For_i(0, loop_n, 1):
                emit_startup_dmas()
                body()

        stack.close()

    nc.compile()
    return nc


def _get_program():
    if "nc" not in _PROGRAM_CACHE:
        _PROGRAM_CACHE["nc"] = _build_program()
    return _PROGRAM_CACHE["nc"]


def _host_consts(position_ids_row):
    inv_freq = 1.0 / (ROPE_THETA ** (np.arange(0, HD, 2, dtype=np.float32) / HD))
    t = position_ids_row.astype(np.float32)
    freqs = t[None, :] * inv_freq[(np.arange(P) % (HD // 2))][:, None]  # [128, S]
    cosT = np.cos(freqs).astype(np.float32)
    sinT = np.sin(freqs).astype(np.float32)
    return np.ascontiguousarray(cosT), np.ascontiguousarray(sinT)


def _make_r2t():
    R = np.zeros((HD, HD), dtype=np.float32)
    for j in range(HD // 2):
        R[j, j + HD // 2] = -1.0
        R[j + HD // 2, j] = 1.0
    R2 = np.zeros((P, P), dtype=np.float32)
    R2[:HD, :HD] = R
    R2[HD:, HD:] = R
    return np.ascontiguousarray(R2.T)


def _make_ones2():
    o = np.zeros((2, P), dtype=np.float32)
    o[0, 0:HD] = 1.0
    o[1, HD:2 * HD] = 1.0
    return o


def _make_masks():
    # masks[p, d*SC + jj] = 1 if jj >= p + d*128 (causal, qs >= ks)
    m = np.zeros((P, 4, SC), dtype=np.float32)
    jj = np.arange(SC)[None, None, :]
    p = np.arange(P)[:, None, None]
    d = np.arange(4)[None, :, None]
    m[:] = (jj >= p + d * P).astype(np.float32)
    return np.ascontiguousarray(m.reshape(P, 4 * SC))


def kernel(**inputs):
    from concourse.bass_utils import run_bass_kernel_spmd

    hs = np.asarray(inputs["hidden_states"], dtype=np.float32)
    wq = np.asarray(inputs["wq"], dtype=np.float32)
    wk = np.asarray(inputs["wk"], dtype=np.float32)
    wv = np.asarray(inputs["wv"], dtype=np.float32)
    wo = np.asarray(inputs["wo"], dtype=np.float32)
    pos = np.asarray(inputs["position_ids"])

    nc = _get_program()

    wqT = np.ascontiguousarray(wq.T)
    wkT = np.ascontiguousarray(wk.T)
    wvT = np.ascontiguousarray(wv.T)
    woT = np.ascontiguousarray(wo.T)
    r2t = _make_r2t()
    masks = _make_masks()
    onesv = np.ones((1, P), dtype=np.float32)
    vones = np.ones((P, HEADS_PER_CORE), dtype=np.float32)
    ones2 = _make_ones2()

    hsT = [np.ascontiguousarray(hs[b].T) for b in range(B)]
    tables = [_host_consts(pos[b]) for b in range(B)]

    in_maps = []
    for core in range(8):
        b, g = divmod(core, GROUPS)
        csl = slice(g * C, (g + 1) * C)
        cosT, sinT = tables[b]
        in_maps.append(dict(
            hsT=hsT[b],
            wqT=np.ascontiguousarray(wqT[:, csl]),
            wkT=np.ascontiguousarray(wkT[:, csl]),
            wvT=np.ascontiguousarray(wvT[:, csl]),
            woT=np.ascontiguousarray(woT[csl, :]),
            cosT=cosT, sinT=sinT, masks=masks, r2t=r2t, onesv=onesv,
            vones=vones, ones2=ones2,
        ))

    res = run_bass_kernel_spmd(nc, in_maps, core_ids=list(range(8)), trace=TRACE)
    global LAST_EXEC_NS
    LAST_EXEC_NS = res.exec_time_ns
    out = np.zeros((B, S, H), dtype=np.float32)
    for core in range(8):
        b = core // GROUPS
        out[b] += res.results[core]["out"]
    return out
